# revision 24
# baseline (speedup 1.0000x reference)
"""3-layer GCN (PyG GCNConv-style) Bass/Trainium2 kernel, 8-way SPMD.

v2 strategy (vs v1): layer 1 is computed aggregate-first — x is replicated
on every core, so the L1 "table" (dinv-prescaled x in the same A/B layout
the AllGathers use) is staged by the host. This removes the L1 AllGather
(the largest collective) and the serial phase-0 GEMM: gathers start at t=0.

  - Core c owns node rows [c*6250, (c+1)*6250).
  - Phase 0 (L1): gather x~ = dinv*x rows per dst-sorted edge schedule,
    aggregate per 128-node window with one-hot "selection" matmuls in PSUM,
    self-loop via identity matmul vs host-staged x~_own windows; then
    y = dinv*psum; z1 = relu(y@W1 [+b1]); h~2 = dinv*(z1@W2) -> table0;
    AllGather table0 in two chunks (A fires mid-phase).
  - Phase 1 (L2): gather h~2, aggregate, z2 = relu(dinv*psum [+b2]),
    h~3 = dinv*(z2@W3) -> table1, AllGather.
  - Phase 2 (L3): gather h~3, aggregate, z3 = relu(dinv*psum) -> finals.
  - Finals: global sum via AllReduce, z/sum -> tanh^2 -> row L2 normalize.
  - All tables / messages / weights bf16 (PSUM accumulates fp32).
  - Schedule identical on all cores -> single NEFF; per-core data lives in
    the gather-index / dst-local metadata / x_own input tensors.
"""

import numpy as np
import ml_dtypes

BF16 = ml_dtypes.bfloat16

# ---- problem constants (hardcoded per contest contract) ----
N = 50000
F0, F1, F2, F3 = 512, 512, 256, 128
NCORES = 8
OWN = N // NCORES            # 6250 rows per core
WIN = 128
NW = (OWN + WIN - 1) // WIN  # 49 windows
OWN_PAD = NW * WIN           # 6272
HALFR = 18 * WIN             # 2304: chunk-A rows/core. A (18 windows) is
# processed LAST so only its small AllGather gates the next phase; the big
# B chunk (31 windows) AllGathers mid-phase. 8*max(HALFR,OWN-HALFR) < 32768.
WORDER = list(range(18, NW)) + list(range(18))   # window processing order
TBLA = NCORES * HALFR        # 31744 rows in table A (< 32768, int16 ok)
TBLB = NCORES * (OWN - HALFR)  # 18256 rows in table B
GW = (F0, F2, F3)            # gathered message width per phase
GL = (16, 32, 32)            # gather tiles per dma_gather call, per phase
CALL = 128                   # stream padding granularity (tail calls trimmed)
SENT = 65000.0               # dst_local sentinel -> never matches iota 0..127
EPS = 1e-12

_BUILD_CACHE = {}


# --------------------------------------------------------------------------
# host-side schedule construction (pure index bookkeeping)
# --------------------------------------------------------------------------

def _build_schedule(src, dst):
    """Returns (sched, per_core) where sched is core-independent."""
    src = src.astype(np.int64)
    dst = dst.astype(np.int64)
    core = dst // OWN
    win = (dst % OWN) // WIN
    r = src % OWN
    chunk = (r >= HALFR).astype(np.int64)

    key = (core * NW + win) * 2 + chunk
    order = np.argsort(key, kind="stable")
    counts = np.bincount(key, minlength=NCORES * NW * 2).reshape(NCORES, NW, 2)
    R = counts.max(axis=0)                      # [NW, 2] padded run lengths
    wo = np.array(WORDER)
    pos = np.zeros((NW, 2), np.int64)           # start position of run (w,c)
    pos[wo[1:], 0] = np.cumsum(R[wo[:-1], 0])
    pos[wo[1:], 1] = np.cumsum(R[wo[:-1], 1])
    slen = R.sum(axis=0)                        # [2] stream lengths
    L = ((slen + CALL - 1) // CALL) * CALL      # padded to gather-call multiple

    # window-of-position per stream (pads extend each run; tail -> -1)
    wof = []
    for c in (0, 1):
        a = np.full(L[c], -1, np.int64)
        a[: slen[c]] = np.repeat(wo, R[wo, c])
        wof.append(a)

    # pair list: (chunk, tile, meta_col) grouped per window
    window_pairs = [[] for _ in range(NW)]
    mcol = 0
    for w in WORDER:
        for c in (0, 1):
            if R[w, c] == 0:
                continue
            t0 = pos[w, c] // 128
            t1 = (pos[w, c] + R[w, c] - 1) // 128
            for t in range(t0, t1 + 1):
                window_pairs[w].append((c, t, mcol))
                mcol += 1
    TP = mcol

    # per-core gather index streams + meta columns
    per_core = []
    for cc in range(NCORES):
        idx_streams = [np.zeros(L[c], np.int64) for c in (0, 1)]
        dstl_streams = [np.full(L[c], SENT, np.float32) for c in (0, 1)]
        for c in (0, 1):
            sel = order[(core[order] == cc) & (chunk[order] == c)]  # by window
            cnt = counts[cc, :, c]
            starts = pos[:, c]
            within = np.arange(sel.shape[0]) - np.repeat(
                np.concatenate([[0], np.cumsum(cnt[:-1])]), cnt
            )
            p = np.repeat(starts, cnt) + within
            rr = src[sel] % OWN
            sz = (HALFR, OWN - HALFR)[c]
            tbl_row = (src[sel] // OWN) * sz + (rr - c * HALFR)
            idx_streams[c][p] = tbl_row
            dstl_streams[c][p] = (dst[sel] % OWN - win[sel] * WIN).astype(np.float32)
            assert tbl_row.max(initial=0) < NCORES * sz

        meta = np.full((128, TP), SENT, np.float32)
        for w in WORDER:
            for c, t, m in window_pairs[w]:
                seg_w = wof[c][t * 128:(t + 1) * 128]
                seg_d = dstl_streams[c][t * 128:(t + 1) * 128]
                meta[:, m] = np.where(seg_w == w, seg_d, SENT)

        imgs = []
        for c in (0, 1):
            a = idx_streams[c].astype(np.int16)
            img = a.reshape(-1, 16).T.copy()          # [16, L/16]
            img = np.tile(img, (8, 1))                # replicate across groups
            imgs.append(np.ascontiguousarray(img))
        per_core.append({"idxA": imgs[0], "idxB": imgs[1],
                         "meta": meta.astype(BF16)})

    sched = {
        "window_pairs": window_pairs,
        "L": [int(L[0]), int(L[1])],
        "TP": TP,
    }
    return sched, per_core


# --------------------------------------------------------------------------
# device kernel builder
# --------------------------------------------------------------------------

def _build_nc(sched, has_bias):
    import concourse.bacc as bacc
    import concourse.mybir as mybir
    import concourse.tile as tile

    f32 = mybir.dt.float32
    bf16 = mybir.dt.bfloat16
    i16 = mybir.dt.int16
    AF = mybir.ActivationFunctionType
    ALU = mybir.AluOpType
    X = mybir.AxisListType.X
    RG = [list(range(NCORES))]

    LA, LB = sched["L"]
    TP = sched["TP"]
    window_pairs = sched["window_pairs"]

    nc = bacc.Bacc("TRN2", target_bir_lowering=False, debug=False,
                   num_devices=NCORES, num_swdge_queues=4)

    xA_t = nc.dram_tensor("xA", [TBLA, F0], bf16, kind="ExternalInput")
    xB_t = nc.dram_tensor("xB", [TBLB, F0], bf16, kind="ExternalInput")
    xown_t = nc.dram_tensor("xown", [OWN_PAD, F0], bf16, kind="ExternalInput")
    idxA_t = nc.dram_tensor("idxA", [128, LA // 16], i16, kind="ExternalInput")
    idxB_t = nc.dram_tensor("idxB", [128, LB // 16], i16, kind="ExternalInput")
    meta_t = nc.dram_tensor("meta", [128, TP], bf16, kind="ExternalInput")
    dinv_t = nc.dram_tensor("dinv_img", [128, NW], f32, kind="ExternalInput")
    ar_t = nc.dram_tensor("arange4", [128, 512], bf16, kind="ExternalInput")
    id_t = nc.dram_tensor("ident", [128, 128], bf16, kind="ExternalInput")
    w1_t = nc.dram_tensor("W1", [F0, F1], bf16, kind="ExternalInput")
    w2_t = nc.dram_tensor("W2", [F1, F2], bf16, kind="ExternalInput")
    w3_t = nc.dram_tensor("W3", [F2, F3], bf16, kind="ExternalInput")
    if has_bias:
        b1_t = nc.dram_tensor("b1", [1, F1], bf16, kind="ExternalInput")
        b2_t = nc.dram_tensor("b2", [1, F2], bf16, kind="ExternalInput")
        b3_t = nc.dram_tensor("b3", [1, F3], bf16, kind="ExternalInput")
        sqd_t = nc.dram_tensor("sqrtdeg", [1, OWN_PAD], bf16, kind="ExternalInput")
    out_t = nc.dram_tensor("out", [OWN, F3], f32, kind="ExternalOutput")

    with tile.TileContext(nc) as tc:
        with (
            tc.tile_pool(name="dram", bufs=1, space="DRAM") as dram,
            tc.tile_pool(name="const", bufs=1) as cst,
            tc.tile_pool(name="sb", bufs=2) as sb,
            tc.tile_pool(name="spool", bufs=6) as sp,
            tc.tile_pool(name="ps", bufs=2, space="PSUM") as ps,
            tc.tile_pool(name="ps1", bufs=1, space="PSUM") as ps1,
        ):
            # ---- resident constants ----
            ar_sb = cst.tile([128, 512], bf16)
            nc.sync.dma_start(ar_sb[:], ar_t.ap())
            id_sb = cst.tile([128, 128], bf16)
            nc.sync.dma_start(id_sb[:], id_t.ap())
            dinv_sb = cst.tile([128, NW], f32)
            nc.sync.dma_start(dinv_sb[:], dinv_t.ap())
            meta_sb = cst.tile([128, TP], bf16)
            nc.sync.dma_start(meta_sb[:], meta_t.ap())
            idx_sb = []
            for name, t_, Lc in (("ia", idxA_t, LA), ("ib", idxB_t, LB)):
                tl = cst.tile([128, Lc // 16], i16, name=name)
                nc.sync.dma_start(tl[:], t_.ap())
                idx_sb.append(tl)
            w_sb = []
            for name, t_, fi, fo in (("w1", w1_t, F0, F1), ("w2", w2_t, F1, F2),
                                     ("w3", w3_t, F2, F3)):
                kt = fi // 128
                tl = cst.tile([128, kt * fo], bf16, name=name)
                nc.sync.dma_start(
                    tl[:].rearrange("p (k f) -> p k f", k=kt),
                    t_.ap().rearrange("(k p) f -> p k f", p=128))
                w_sb.append(tl)
            b_sb = []
            sqd_sb = None
            ones_sb = None
            if has_bias:
                for name, t_, fo in (("b1s", b1_t, F1), ("b2s", b2_t, F2),
                                     ("b3s", b3_t, F3)):
                    tl = cst.tile([1, fo], bf16, name=name)
                    nc.sync.dma_start(tl[:], t_.ap())
                    b_sb.append(tl)
                sqd_sb = cst.tile([1, OWN_PAD], bf16)
                nc.sync.dma_start(sqd_sb[:], sqd_t.ap())
                ones_sb = cst.tile([1, 128], bf16)
                nc.vector.memset(ones_sb[:], 1.0)
            z_big = cst.tile([128, NW * F3], f32)
            zpart = cst.tile([128, NW], f32)

            # ---- DRAM intermediates (tables 0=h~2 width F2, 1=h~3 width F3) --
            TWD = (F2, F3)
            agA = [dram.tile([HALFR, f], bf16, name=f"agA{i}")
                   for i, f in enumerate(TWD)]
            agB = [dram.tile([OWN_PAD - HALFR, f], bf16, name=f"agB{i}")
                   for i, f in enumerate(TWD)]
            tblA = [dram.tile([TBLA, f], bf16, name=f"tA{i}", addr_space="Shared")
                    for i, f in enumerate(TWD)]
            tblB = [dram.tile([TBLB, f], bf16, name=f"tB{i}", addr_space="Shared")
                    for i, f in enumerate(TWD)]
            BW = HALFR // 128           # full windows in the A chunk
            BCUT = HALFR - BW * 128     # rows of the boundary window in A
            AGW = BW if BCUT > 0 else BW - 1   # window whose write completes A

            def dinv_col(w):
                return dinv_sb[:, w:w + 1]

            def write_h(hb, w, ti):
                a, b = agA[ti], agB[ti]
                if w < BW:
                    nc.sync.dma_start(a[w * 128:(w + 1) * 128, :], hb[:])
                elif w == BW and BCUT > 0:
                    nc.sync.dma_start(a[BW * 128:HALFR, :], hb[:BCUT, :])
                    nc.sync.dma_start(b[0:128 - BCUT, :], hb[BCUT:, :])
                else:
                    o = w * 128 - HALFR
                    nc.sync.dma_start(b[o:o + 128, :], hb[:])

            def read_own(ob, w, ti):
                a, b = agA[ti], agB[ti]
                if w < BW:
                    nc.sync.dma_start(ob[:], a[w * 128:(w + 1) * 128, :])
                elif w == BW and BCUT > 0:
                    nc.sync.dma_start(ob[:BCUT, :], a[BW * 128:HALFR, :])
                    nc.sync.dma_start(ob[BCUT:, :], b[0:128 - BCUT, :])
                else:
                    o = w * 128 - HALFR
                    nc.sync.dma_start(ob[:], b[o:o + 128, :])

            def emit_agA(ti):
                nc.gpsimd.collective_compute(
                    "AllGather", ALU.bypass, replica_groups=RG,
                    ins=[agA[ti][:].opt()], outs=[tblA[ti][:].opt()])

            def emit_agB(ti):
                nc.gpsimd.collective_compute(
                    "AllGather", ALU.bypass, replica_groups=RG,
                    ins=[agB[ti][0:OWN - HALFR, :].opt()],
                    outs=[tblB[ti][:].opt()])

            # ---- GEMM for one 128-row block (node-major in and out);
            # writes dinv*(blk@W) to table ti ----
            def gemm_block(blk, w, fi, fo, wsb, ti):
                kt = fi // 128
                psg = ps.tile([128, fo], f32, name="psg", tag="psg")
                for k in range(kt):
                    pst = ps.tile([128, 128], bf16, name="pst", tag="pst")
                    nc.tensor.transpose(pst[:], blk[:, k * 128:(k + 1) * 128],
                                        id_sb[:])
                    hT = sb.tile([128, 128], bf16, name="hT", tag="hT", bufs=8)
                    nc.scalar.copy(hT[:], pst[:])
                    nc.tensor.matmul(psg[:], lhsT=hT[:],
                                     rhs=wsb[:, k * fo:(k + 1) * fo],
                                     start=(k == 0), stop=(k == kt - 1))
                hb = sb.tile([128, fo], bf16, name="hb", tag="hb")
                nc.scalar.mul(hb[:], psg[:], dinv_col(w))
                write_h(hb, w, ti)

            # ---- phases ----
            # Gathers on 4 SWDGE queues (chunk A -> q0/q2, chunk B -> q1/q3).
            lane_sems = tc.sems.swdge_block()
            gcount = [0]  # global dma_gather emission counter:
            # DMASW lane = count%8; queue = count%4 keeps each lane
            # locked to a single SWDGE queue (ucode requirement).
            LOOK = 3
            with tc.tile_pool(name="mpool", bufs=6) as mp:
                for ph in range(3):
                    fo = GW[ph]
                    if ph == 0:
                        tviews = (xA_t.ap(), xB_t.ap())
                    else:
                        tviews = (tblA[ph - 1][:], tblB[ph - 1][:])

                    live = [{}, {}]
                    prepped = [0, 0]
                    triggered = [0, 0]
                    s4_live = {}
                    s4_next = [0]
                    Gl = GL[ph]
                    CALLl = Gl * 128
                    ncalls = [(LA + CALLl - 1) // CALLl,
                              (LB + CALLl - 1) // CALLl]

                    def prep(c, hi, fo=fo, tviews=tviews, live=live,
                             prepped=prepped, Gl=Gl, CALLl=CALLl,
                             ncalls=ncalls):
                        hi = min(hi, ncalls[c] - 1)
                        while prepped[c] <= hi:
                            ci = prepped[c]
                            nrow = min(CALLl, (LA, LB)[c] - ci * CALLl)
                            nt = nrow // 128
                            m = mp.tile([128, Gl * fo], bf16, name=f"m{c}",
                                        tag=f"m{c}")
                            nc.gpsimd.dma_gather(
                                m[:, :nt * fo].rearrange(
                                    "p (t f) -> p t f", f=fo),
                                tviews[c],
                                idx_sb[c][:, ci * (CALLl // 16):
                                          ci * (CALLl // 16) + nrow // 16],
                                nrow, nrow, fo,
                                queue_num=gcount[0] % 4)
                            gcount[0] += 1
                            live[c][ci] = m
                            prepped[c] += 1

                    def ensure(c, t, live=live, prepped=prepped,
                               triggered=triggered, Gl=Gl, ncalls=ncalls,
                               prep=prep):
                        call = t // Gl
                        if triggered[c] <= call:
                            prep(c, call + LOOK)
                            triggered[c] = prepped[c]
                        return live[c][call]

                    def ensure_s4(mcol, s4_live=s4_live, s4_next=s4_next):
                        b = mcol // 4
                        while s4_next[0] <= b:
                            bi = s4_next[0]
                            m0 = bi * 4
                            nb = min(4, TP - m0)
                            s4 = sp.tile([128, 512], bf16, name="s4", tag="s4")
                            nc.vector.tensor_tensor(
                                out=s4[:].rearrange(
                                    "p (a f) -> p a f", a=4)[:, :nb, :],
                                in0=ar_sb[:].rearrange(
                                    "p (a f) -> p a f", a=4)[:, :nb, :],
                                in1=meta_sb[:, m0:m0 + nb].to_broadcast(
                                    [128, nb, 128]),
                                op=ALU.is_equal)
                            s4_live[bi] = s4
                            s4_next[0] += 1
                        return s4_live[b]

                    # desc-gen warm-up: queue LOOK+1 calls deep on both
                    # streams before the first window consumes anything.
                    prep(0, LOOK)
                    prep(1, LOOK)

                    for w in WORDER:
                        psw = ps.tile([128, fo], f32, name="psw", tag="psw")
                        first = True
                        for c, t, mcolv in window_pairs[w]:
                            m = ensure(c, t)
                            s4 = ensure_s4(mcolv)
                            j = mcolv % 4
                            sl = t % Gl
                            nc.tensor.matmul(psw[:],
                                             lhsT=s4[:, j * 128:(j + 1) * 128],
                                             rhs=m[:, sl * fo:(sl + 1) * fo],
                                             start=first, stop=False)
                            first = False
                        ob = sb.tile([128, fo], bf16, name="ob", tag="ob")
                        if ph == 0:
                            nc.sync.dma_start(
                                ob[:], xown_t.ap()[w * 128:(w + 1) * 128, :])
                        else:
                            read_own(ob, w, ph - 1)
                        # self-loop; for ph>0 with bias the extra rank-1
                        # matmul carries sqrtdeg*b into the psum.
                        stop_here = (ph == 0) or not has_bias
                        nc.tensor.matmul(psw[:], lhsT=id_sb[:], rhs=ob[:],
                                         start=first, stop=stop_here)
                        if ph > 0 and has_bias:
                            nc.tensor.matmul(
                                psw[:],
                                lhsT=sqd_sb[0:1, w * 128:(w + 1) * 128],
                                rhs=b_sb[ph][0:1, :], start=False, stop=True)
                        if ph == 0:
                            # y = dinv*psw; z1 = relu(y@W1 + b1); table0
                            y = sb.tile([128, F0], bf16, name="yag", tag="yag")
                            nc.scalar.mul(y[:], psw[:], dinv_col(w))
                            kt = F0 // 128
                            psg = ps.tile([128, F1], f32, name="psg1",
                                          tag="psg")
                            for k in range(kt):
                                pst = ps.tile([128, 128], bf16, name="pst",
                                              tag="pst")
                                nc.tensor.transpose(
                                    pst[:], y[:, k * 128:(k + 1) * 128],
                                    id_sb[:])
                                hT = sb.tile([128, 128], bf16, name="hT",
                                             tag="hT", bufs=8)
                                nc.scalar.copy(hT[:], pst[:])
                                nc.tensor.matmul(
                                    psg[:], lhsT=hT[:],
                                    rhs=w_sb[0][:, k * F1:(k + 1) * F1],
                                    start=(k == 0),
                                    stop=(k == kt - 1) and not has_bias)
                            if has_bias:
                                nc.tensor.matmul(psg[:], lhsT=ones_sb[0:1, :],
                                                 rhs=b_sb[0][0:1, :],
                                                 start=False, stop=True)
                            z1 = sb.tile([128, F1], bf16, name="z1", tag="z1")
                            nc.scalar.activation(z1[:], psg[:], AF.Relu,
                                                 bias=0.0, scale=1.0)
                            gemm_block(z1, w, F1, F2, w_sb[1], 0)
                            if w == NW - 1:
                                emit_agB(0)
                        elif ph == 1:
                            hb2 = sb.tile([128, F2], bf16, name="hbw",
                                          tag="hbw")
                            nc.scalar.activation(hb2[:], psw[:], AF.Relu,
                                                 bias=0.0, scale=dinv_col(w))
                            gemm_block(hb2, w, F2, F3, w_sb[2], 1)
                            if w == NW - 1:
                                emit_agB(1)
                        else:
                            nc.scalar.activation(
                                z_big[:, w * F3:(w + 1) * F3], psw[:],
                                AF.Relu, bias=0.0, scale=dinv_col(w))
                            nc.vector.reduce_sum(
                                zpart[:, w:w + 1],
                                z_big[:, w * F3:(w + 1) * F3], axis=X)
                    if ph < 2:
                        emit_agA(ph)

            # ---- finals ----
            zsum = cst.tile([128, 1], f32)
            nc.vector.reduce_sum(zsum[:], zpart[:], axis=X)
            ones = cst.tile([128, 1], f32)
            nc.vector.memset(ones[:], 1.0)
            pss = ps1.tile([128, 16], f32)
            nc.tensor.matmul(pss[0:1, 0:1], lhsT=ones[:], rhs=zsum[:],
                             start=True, stop=True)
            tot_sb = cst.tile([1, 16], f32)
            nc.vector.memset(tot_sb[:], 0.0)
            nc.scalar.copy(tot_sb[0:1, 0:1], pss[0:1, 0:1])
            ar_in = dram.tile([1, 16], f32)
            ar_out = dram.tile([1, 16], f32, addr_space="Shared")
            nc.sync.dma_start(ar_in[:], tot_sb[:])
            nc.gpsimd.collective_compute(
                "AllReduce", ALU.add, replica_groups=RG,
                ins=[ar_in[:].opt()], outs=[ar_out[:].opt()])
            tot2 = cst.tile([1, 16], f32)
            nc.sync.dma_start(tot2[:], ar_out[:])
            tot_bc = cst.tile([128, 1], f32)
            nc.gpsimd.partition_broadcast(tot_bc[:], tot2[0:1, 0:1],
                                          channels=128)
            inv_tot = cst.tile([128, 1], f32)
            nc.vector.reciprocal(inv_tot[:], tot_bc[:])

            # z/sum -> tanh -> ^2 ; then row L2 norm, all column-batched.
            # Processed in two window halves so scalar tanh of half k+1
            # overlaps DVE squares/norms of half k, and each half's output
            # DMA starts early. Ping-pong z_big <-> scr to bound SBUF.
            scr = cst.tile([128, NW * F3], f32)
            s4s = cst.tile([128, NW], f32)
            nmr = cst.tile([128, NW], f32)
            rinv = cst.tile([128, NW], f32)
            for w0, w1 in ((0, 25), (25, NW)):
                nwn = w1 - w0
                sl = slice(w0 * F3, w1 * F3)
                nc.scalar.activation(scr[:, sl], z_big[:, sl], AF.Tanh,
                                     bias=0.0, scale=inv_tot[:])
                nc.vector.tensor_tensor(out=z_big[:, sl], in0=scr[:, sl],
                                        in1=scr[:, sl], op=ALU.mult)
                nc.vector.tensor_tensor(out=scr[:, sl], in0=z_big[:, sl],
                                        in1=z_big[:, sl], op=ALU.mult)
                nc.vector.reduce_sum(
                    s4s[:, w0:w1].rearrange("p w -> p w ()"),
                    scr[:, sl].rearrange("p (w f) -> p w f", w=nwn), axis=X)
                nc.scalar.sqrt(nmr[:, w0:w1], s4s[:, w0:w1])
                nc.vector.reciprocal(rinv[:, w0:w1], nmr[:, w0:w1])
                nc.vector.tensor_scalar_min(rinv[:, w0:w1], rinv[:, w0:w1],
                                            1.0 / EPS)
                nc.vector.tensor_tensor(
                    out=scr[:, sl].rearrange("p (w f) -> p w f", w=nwn),
                    in0=z_big[:, sl].rearrange("p (w f) -> p w f", w=nwn),
                    in1=rinv[:, w0:w1].to_broadcast([128, nwn, F3]),
                    op=ALU.mult)
                wfull = min(w1, NW - 1)
                nc.sync.dma_start(
                    out_t.ap()[w0 * 128:wfull * 128, :].rearrange(
                        "(w p) f -> p w f", p=128),
                    scr[:, sl].rearrange(
                        "p (w f) -> p w f", w=nwn)[:, 0:wfull - w0, :])
                if w1 == NW:
                    tail = OWN - (NW - 1) * 128
                    nc.sync.dma_start(
                        out_t.ap()[(NW - 1) * 128:OWN, :],
                        scr[0:tail, (NW - 1) * F3:NW * F3])


    nc.compile()

    # The Tile scheduler reorders instructions, then assigns DMASW completion
    # lanes round-robin in *module* order; ucode locks each lane sem to one
    # SWDGE queue. Re-derive queue_num from the assigned lane post-compile so
    # the lane<->queue pairing is consistent regardless of scheduling.
    import re as _re
    import concourse.mybir as _mybir

    def _walk(bb):
        for inst in bb.instructions:
            yield inst
            for sub in (getattr(inst, "body_bb", None),
                        getattr(inst, "else_bb", None)):
                if sub is not None:
                    yield from _walk(sub)

    for _bb in nc.m.functions[0].blocks:
        for _inst in _walk(_bb):
            if isinstance(_inst, _mybir.InstDMAGatherAnt):
                _si = _inst.sync_info
                for _u in (_si.on_update if _si is not None else []):
                    _m = _re.match(r"DMASW(\d+)_", _u.ant_name or "")
                    if _m:
                        _inst.queue_num = int(_m.group(1)) % 4
                        break
    return nc


# --------------------------------------------------------------------------
# entry point
# --------------------------------------------------------------------------

def _prepare(x, edge_index, W1, b1, W2, b2, W3, b3):
    x = np.ascontiguousarray(np.asarray(x, dtype=np.float32))
    ei = np.asarray(edge_index)
    src = np.ascontiguousarray(ei[0]).astype(np.int64)
    dst = np.ascontiguousarray(ei[1]).astype(np.int64)
    W1 = np.ascontiguousarray(np.asarray(W1, np.float32)).astype(BF16)
    W2 = np.ascontiguousarray(np.asarray(W2, np.float32)).astype(BF16)
    W3 = np.ascontiguousarray(np.asarray(W3, np.float32)).astype(BF16)
    b1 = np.asarray(b1, np.float32)
    b2 = np.asarray(b2, np.float32)
    b3 = np.asarray(b3, np.float32)
    has_bias = bool(np.any(b1) or np.any(b2) or np.any(b3))

    deg = (np.bincount(dst, minlength=N) + 1.0).astype(np.float32)
    dinv = (1.0 / np.sqrt(deg.astype(np.float64))).astype(np.float32)

    # Degree-balanced relabel: snake-deal nodes (by in-degree, descending)
    # across the 8 cores so each window's 8 per-core bins carry near-equal
    # edge counts. The per-(window,chunk) padded run length is a max over
    # cores, so balancing cuts schedule padding from ~12% to ~5%.
    order_nodes = np.argsort(-deg, kind="stable")
    j = np.arange(N)
    grp = j // NCORES
    within = j % NCORES
    core_of = np.where(grp % 2 == 0, within, NCORES - 1 - within)
    # scatter the degree-sorted groups across positions (fixed seed) so each
    # window draws uniformly from all degree bands -> steady per-window load
    pos_of = np.random.default_rng(0).permutation(OWN)[grp]
    perm = np.empty(N, np.int64)                      # node -> slot
    perm[order_nodes] = core_of * OWN + pos_of
    node_at = np.empty(N, np.int64)                   # slot -> node
    node_at[perm] = np.arange(N)
    src = perm[src]
    dst = perm[dst]
    deg = deg[node_at]                                # now indexed by slot
    dinv = dinv[node_at]

    ck = hash((src.tobytes(), dst.tobytes(), has_bias))
    if ck in _BUILD_CACHE:
        nc, sched, per_core = _BUILD_CACHE[ck]
    else:
        sched, per_core = _build_schedule(src, dst)
        nc = _build_nc(sched, has_bias)
        _BUILD_CACHE[ck] = (nc, sched, per_core)

    # dinv-prescaled x in the AllGather table layout (A: rows [0,HALFR) of
    # each core, B: rows [HALFR,OWN)), shared content for all cores.
    xt = (x[node_at] * dinv[:, None]).astype(BF16)    # [N, F0], slot-ordered
    xr = xt.reshape(NCORES, OWN, F0)
    xA = np.ascontiguousarray(xr[:, :HALFR].reshape(TBLA, F0))
    xB = np.ascontiguousarray(xr[:, HALFR:].reshape(TBLB, F0))

    arange4 = np.tile(np.arange(128, dtype=np.float32)[None, :],
                      (128, 4)).astype(BF16)
    ident = np.eye(128, dtype=np.float32).astype(BF16)
    in_maps = []
    for c in range(NCORES):
        lo = c * OWN
        xown = np.zeros((OWN_PAD, F0), BF16)
        xown[:OWN] = xt[lo:lo + OWN]
        dv = np.ones(OWN_PAD, np.float32)
        dv[:OWN] = dinv[lo:lo + OWN]
        dinv_img = np.ascontiguousarray(dv.reshape(NW, 128).T)
        m = {
            "xA": xA,
            "xB": xB,
            "xown": xown,
            "idxA": per_core[c]["idxA"],
            "idxB": per_core[c]["idxB"],
            "meta": per_core[c]["meta"],
            "dinv_img": dinv_img,
            "arange4": arange4,
            "ident": ident,
            "W1": W1, "W2": W2, "W3": W3,
        }
        if has_bias:
            sq = np.zeros((1, OWN_PAD), np.float32)
            sq[0, :OWN] = np.sqrt(deg[lo:lo + OWN])
            m["b1"] = b1.reshape(1, F1).astype(BF16)
            m["b2"] = b2.reshape(1, F2).astype(BF16)
            m["b3"] = b3.reshape(1, F3).astype(BF16)
            m["sqrtdeg"] = sq.astype(BF16)
        in_maps.append(m)
    return nc, in_maps, perm


def kernel(x, edge_index, W1, b1, W2, b2, W3, b3):
    from concourse.bass_utils import run_bass_kernel_spmd

    nc, in_maps, perm = _prepare(x, edge_index, W1, b1, W2, b2, W3, b3)
    res = run_bass_kernel_spmd(nc, in_maps, core_ids=list(range(NCORES)),
                               **_RUN_KWARGS)
    global _LAST
    _LAST = res
    out = np.concatenate([res.results[c]["out"] for c in range(NCORES)], axis=0)
    return np.ascontiguousarray(out[perm])


# test.py hooks (harness never touches these)
_RUN_KWARGS = {}
_LAST = None
